# revision 64
# baseline (speedup 1.0000x reference)
"""Distributed Trainium2 kernel for nn_Attention_14697378086932.

Head-sharded (tensor-parallel) multi-head attention over 8 NeuronCores:
each core computes 2 of the 16 heads end-to-end.

Per core (all matmul stationaries are full 128-wide so the PE clock
gate stays at 8/8):
  - x^T is pre-chunked on the host so every 2MB input tile DMA is
    128 x 16KB contiguous runs (~430 GB/s vs ~250 for 2KB runs); the
    first chunk's DMA is issued ahead of the small weight loads.
  - QKV projections with stationary-operand reuse: the hidden-chunk
    loop sits outside a 4-token-chunk group, so each weight chunk is
    LDWEIGHTS'd once per 4 matmuls (LDWEIGHTS is ~100-230ns of serial
    PE time on HW).
  - rotary: only global channels 0..63 are rotated (reference quirk);
    cores 1..7 receive cos=1/sin=0.  rotate_half is a permutation
    matrix on the PE; rope math runs in f32 and lands as one bf16
    round in the attention operands (Qb / KzA / KzB).
  - V transposes run in bf16 (1 cycle/row), 4 key-chunks per PSUM
    tile, moved into the PV stationary with one strided copy per head.
  - attention per (batch, 1024-q block, local head), flash-style over
    128-key chunks: S^T = Kz Qb^T in bf16 (f32r matmuls cost ~2x bf16
    on HW), P^T = exp(S^T) on ScalarE (no max subtraction: logits are
    bounded, f32 exp is safe), O^T = [V | ones]^T P^T in bf16.  The 64
    ones columns replicate the softmax denominator into PSUM rows
    64..127.  The S/exp/PV pipeline carries across block boundaries;
    ScalarE's exp stream (~1.01us per [128,1024] chunk) is the phase
    floor.
  - normalize: two DVE PSUM bounces (O rows, denominator rows), one
    reciprocal_approx_fast over all 64 rows, one multiply -- all
    partition-0-based custom-DVE-safe APs, no partition broadcast.
  - output projection: partial = O_loc @ Wo_c (bf16), two 512-wide
    matmuls per 128-token chunk into rotating PSUM banks, bounced to
    SBUF (DVE, plus idle ScalarE during the drain) and DMA'd out,
    interleaved into later attention blocks.  Dependency-staggered
    dummy matmuls bridge the final normalize window so the HAM clock
    gate never throttles the drain.  Host sums the 8 partials + bo.
"""
import os
import sys

# A crashed load can leave cores in a degraded-clock state that slows
# every later run ~20%; resetting at init restores full speed.
os.environ.setdefault("NEURON_RT_RESET_CORES", "1")

sys.path.insert(0, "/opt/trn_rl_repo")

import numpy as np
import ml_dtypes

import concourse.bass as bass
import concourse.mybir as mybir
from concourse import bacc
from concourse.bass import ts, ds
from concourse.tile import TileContext
from concourse.masks import make_identity
from concourse.bass_utils import run_bass_kernel_spmd

F32 = mybir.dt.float32
F32R = mybir.dt.float32r
BF16 = mybir.dt.bfloat16

P = 128          # partitions / local channels per core
HID = 1024       # hidden
NT = 4096        # total tokens (batch 2 x 2048)
NB = 2048        # tokens per batch
HD = 64          # head dim
N_CORES = 8

# Wide-matmul experiments: a single matmul whose PSUM output spans two
# 2KB banks ([128, 1024] f32).
S_SINGLE = False    # S^T chunk as one 1024-free matmul
PV_SINGLE = False   # PV accumulation as one 1024-free matmul
NORM_V2 = True      # approx-recip normalize (False: baseline recip chain)

_NC_CACHE = None


def build_nc():
    nc = bacc.Bacc("TRN2")

    # x^T pre-chunked on the host: [chunk, p, o, tok] so each 2MB tile
    # DMA is 128 x 16KB contiguous runs (vs 1024 x 2KB from a plain
    # [HID, NT] layout), which reaches near-peak HBM bandwidth.
    xt = nc.declare_dram_parameter("xt", [NT // 512, P, 8, 512], F32R,
                                   isOutput=False)
    wq = nc.declare_dram_parameter("wq", [HID, P], F32R, isOutput=False)
    wk = nc.declare_dram_parameter("wk", [HID, P], F32R, isOutput=False)
    wv = nc.declare_dram_parameter("wv", [HID, P], F32R, isOutput=False)
    wo = nc.declare_dram_parameter("wo", [P, HID], BF16, isOutput=False)
    bia = nc.declare_dram_parameter("bias", [P, 3], F32, isOutput=False)
    cos = nc.declare_dram_parameter("cos", [HD, NT], BF16, isOutput=False)
    sin = nc.declare_dram_parameter("sin", [HD, NT], BF16, isOutput=False)
    rmat = nc.declare_dram_parameter("rmat", [P, P], F32R, isOutput=False)
    out = nc.declare_dram_parameter("out", [NT, HID], F32, isOutput=True)

    wq_r = wq[:].rearrange("(o p) m -> p o m", p=P)      # [128, 8, 128]
    wk_r = wk[:].rearrange("(o p) m -> p o m", p=P)
    wv_r = wv[:].rearrange("(o p) m -> p o m", p=P)

    with TileContext(nc) as tc:
        with tc.tile_pool(name="consts", bufs=1) as consts, \
             tc.tile_pool(name="big", bufs=1) as big, \
             tc.tile_pool(name="xtp", bufs=4) as xtp, \
             tc.tile_pool(name="x0p", bufs=1) as x0p:
            # x chunk DMAs go out first: the DMA issue queue is serial
            # (~0.7us per issue) and the first QKV matmul needs chunk 0,
            # so the small weight loads must not delay it.
            # wq first (the very first matmul's stationary), then
            # chunk 0 in two o-halves so compute starts after 1MB.
            wqs = consts.tile([P, 8, P], F32R)
            nc.sync.dma_start(wqs, wq_r)
            xtts_all = []
            xt0a = x0p.tile([P, 4, 512], F32R, tag="x0a", name="x0a")
            xt0b = x0p.tile([P, 4, 512], F32R, tag="x0b", name="x0b")
            nc.sync.dma_start(xt0a, xt[0, :, 0:4])
            nc.sync.dma_start(xt0b, xt[0, :, 4:8])
            xtts_all.append((xt0a, xt0b))
            wks = consts.tile([P, 8, P], F32R)
            wvs = consts.tile([P, 8, P], F32R)
            nc.sync.dma_start(wks, wk_r)
            nc.sync.dma_start(wvs, wv_r)
            wos = consts.tile([P, HID], BF16)
            nc.sync.dma_start(wos, wo[:])
            bias_t = consts.tile([P, 3], F32)
            nc.sync.dma_start(bias_t, bia[:])
            rmat_t = consts.tile([P, P], F32R)
            nc.sync.dma_start(rmat_t, rmat[:])
            ident = consts.tile([P, P], F32)
            make_identity(nc, ident)
            identb = consts.tile([P, P], BF16)
            make_identity(nc, identb)

            Qb = big.tile([P, NT], BF16)     # Q^T bf16 (rope applied)
            # normalized attention out^T, one tile per (batch, nq-block);
            # the final block's tile is split in half so its output
            # projection can start after the first normalize half
            OtT = []
            for k in range(3):
                ot_k = big.tile([P, 1024], BF16, name=f"Ot{k}")
                OtT.append(ot_k)
            Ot3 = [big.tile([P, 512], BF16, name=f"Ot3{h}")
                   for h in range(2)]
            # Zero-padded per-head K^T in bf16: head A in rows 0..63,
            # head B in rows 64..127.  Full-128 stationary keeps the PE
            # clock gate at 8/8.
            KzA = big.tile([P, NT], BF16)
            KzB = big.tile([P, NT], BF16)
            nc.gpsimd.memset(KzA[HD:P, :], 0.0)
            nc.gpsimd.memset(KzB[0:HD, :], 0.0)
            # V in [token, channel] layout per head per 128-token key
            # chunk: [128 tok, 32 chunks, 64 V | 64 ones].  The 64 ones
            # columns replicate the softmax denominator into PSUM rows
            # 64..127, so normalize needs no partition broadcast.
            VaugA = big.tile([P, 32, P], BF16)
            VaugB = big.tile([P, 32, P], BF16)
            nc.gpsimd.memset(VaugA[:, :, 0:64], 0.0)
            nc.gpsimd.memset(VaugB[:, :, 0:64], 0.0)
            nc.gpsimd.memset(VaugA[:, :, 64:P], 1.0)
            nc.gpsimd.memset(VaugB[:, :, 64:P], 1.0)

            # ---------------- Phase A: QKV projections + rope + V transpose
            # Token chunks run in groups of 4 with the hidden-chunk (o)
            # loop outside the group: each weight chunk is LDWEIGHTS'd
            # once per 4 matmuls.  Q/K/V intermediates live in small
            # per-chunk scratch tiles; only the bf16 attention operands
            # persist.
            with tc.tile_pool(name="scr", bufs=5) as scr, \
                 tc.tile_pool(name="ropet", bufs=2) as ropet, \
                 tc.tile_pool(name="trig", bufs=1) as trig, \
                 tc.tile_pool(name="psA", bufs=1, space="PSUM") as psA, \
                 tc.tile_pool(name="psRT", bufs=2, space="PSUM") as psRT:
                cos_t = trig.tile([HD, NT], BF16)
                sin_t = trig.tile([HD, NT], BF16)
                # chunk 1 is needed ~7us after chunk 0's compute starts;
                # issue it before the trig tables so it isn't 2.3us late.
                for u in range(1, 8):
                    xtt = xtp.tile([P, 8, 512], F32R, tag="xt")
                    nc.sync.dma_start(xtt, xt[u])
                    xtts_all.append(xtt)
                    if u == 1:
                        nc.sync.dma_start(cos_t, cos[:])
                        nc.sync.dma_start(sin_t, sin[:])
                for g in range(2):    # groups of 4 512-token chunks
                    xtts = xtts_all[4 * g:4 * g + 4]
                    scrs = {}
                    for wt, bidx, nm in ((wqs, 0, "q"), (wks, 1, "k"),
                                         (wvs, 2, "v")):
                        pss = [psA.tile([P, 512], F32, tag=f"ps{u}",
                                        name=f"ps{u}")
                               for u in range(4)]
                        for o in range(8):
                            for u in range(4):
                                xta = xtts[u]
                                if isinstance(xta, tuple):
                                    mov = xta[o // 4][:, o % 4]
                                else:
                                    mov = xta[:, o]
                                nc.tensor.matmul(pss[u], wt[:, o], mov,
                                                 start=(o == 0), stop=(o == 7))
                        row = []
                        for u in range(4):
                            # V scratch is bf16 (transposes at 1
                            # cycle/row); Q/K stay f32 so rope keeps
                            # full precision before the one bf16 round.
                            st = scr.tile([P, 512],
                                          BF16 if nm == "v" else F32R,
                                          tag=f"s{nm}", name=f"s{nm}{u}")
                            nc.scalar.activation(
                                st, pss[u],
                                mybir.ActivationFunctionType.Identity,
                                bias=bias_t[:, bidx:bidx + 1])
                            row.append(st)
                        scrs[nm] = row
                    for u in range(4):
                        sl = ts(4 * g + u, 512)
                        for nm, rot_dst, un_dst in (
                                ("q", Qb, Qb), ("k", KzA, KzB)):
                            src = scrs[nm][u]
                            psr = psRT.tile([P, 512], F32, tag="rt")
                            nc.tensor.matmul(psr, rmat_t, src,
                                             start=True, stop=True)
                            tmp = ropet.tile([HD, 512], F32, tag="tmp")
                            nc.vector.tensor_tensor(
                                tmp, psr[0:HD], sin_t[:, sl],
                                mybir.AluOpType.mult)
                            tmp2 = ropet.tile([HD, 512], F32, tag="tmp2")
                            nc.vector.tensor_tensor(
                                tmp2, src[0:HD].bitcast(F32),
                                cos_t[:, sl], mybir.AluOpType.mult)
                            nc.vector.tensor_tensor(
                                rot_dst[0:HD, sl], tmp2, tmp,
                                mybir.AluOpType.add)
                            nc.vector.tensor_copy(
                                un_dst[HD:P, sl], src[HD:P].bitcast(F32))
                        # V transpose into per-head layout: 4 k-chunks
                        # land in one bf16 PSUM tile, then one strided
                        # copy per head moves all 4 into Vaug.
                        kc0 = (4 * g + u) * 4
                        pst = psRT.tile([P, 4, P], BF16, tag="rtb",
                                        name="rtb")
                        for s in range(4):
                            nc.tensor.transpose(
                                pst[:, s, :],
                                scrs["v"][u][:, ts(s, P)],
                                identb)
                        nc.vector.tensor_copy(VaugA[:, kc0:kc0 + 4, 0:HD],
                                              pst[:, :, 0:HD])
                        nc.vector.tensor_copy(VaugB[:, kc0:kc0 + 4, 0:HD],
                                              pst[:, :, HD:P])

            # ---------------- Phase B: attention + output projection
            with tc.tile_pool(name="ptp", bufs=12) as ptp, \
                 tc.tile_pool(name="osb", bufs=3) as osb, \
                 tc.tile_pool(name="nrm", bufs=2) as nrm, \
                 tc.tile_pool(name="spS", bufs=2, space="PSUM") as spS, \
                 tc.tile_pool(name="spO", bufs=1, space="PSUM") as spO, \
                 tc.tile_pool(name="spP", bufs=2, space="PSUM") as spP:

                def oproj_tile(q0, tch, use_act=False):
                    # output projection of one 128-token chunk (both
                    # heads): two 512-wide matmuls into rotating single
                    # PSUM banks so each copy overlaps the next matmul.
                    # During the drain the idle ScalarE takes every other
                    # copy so the DVE isn't the serial bottleneck.
                    t0 = q0 + tch * P
                    if q0 == 3072:
                        lhs = Ot3[tch // 4][:, ts(tch % 4, P)]
                    else:
                        lhs = OtT[q0 // 1024][:, ts(tch, P)]
                    ost = osb.tile([P, HID], F32, tag="ost")
                    for hf in range(2):
                        Pps = spP.tile([P, 512], F32, tag="oproj")
                        nc.tensor.matmul(Pps, lhs, wos[:, ts(hf, 512)],
                                         start=True, stop=True)
                        if use_act and hf == 1:
                            nc.scalar.activation(
                                ost[:, ts(hf, 512)], Pps,
                                mybir.ActivationFunctionType.Identity)
                        else:
                            nc.vector.tensor_copy(ost[:, ts(hf, 512)], Pps)
                    nc.sync.dma_start(out[t0:t0 + P, :], ost)

                def normalize(hlo, q0, Ops, last=False):
                    # O and the (row-replicated) denominators bounce from
                    # PSUM, then one approx-reciprocal over all 64 rows
                    # and one multiply -- all partition-0-based, no
                    # gpsimd.  On the final block the O bounce runs on
                    # the now-idle ScalarE to shorten the DVE chain.
                    osO = nrm.tile([HD, 1024], F32, tag="osO", name="osO")
                    osD = nrm.tile([HD, 1024], F32, tag="osD", name="osD")
                    rc = nrm.tile([HD, 1024], F32, tag="rc", name="rc")

                    def copyO(dst, src):
                        # osD heads the serial DVE chain; on the final
                        # block osO bounces via the idle ScalarE instead.
                        if last:
                            nc.scalar.activation(
                                dst, src,
                                mybir.ActivationFunctionType.Identity)
                        else:
                            nc.vector.tensor_copy(dst, src)

                    if q0 == 3072:
                        # final (b, nqb): per-half chains into the split
                        # OtT so the drain starts after half 0
                        for hh in range(2):
                            sl = ts(hh, 512)
                            nc.vector.tensor_copy(osD[:, sl],
                                                  Ops[HD:P, sl])
                            copyO(osO[:, sl], Ops[0:HD, sl])
                            nc.vector.reciprocal_approx_fast(
                                rc[:, sl], osD[:, sl])
                            nc.vector.tensor_tensor(
                                Ot3[hh][hlo:hlo + HD, :],
                                osO[:, sl], rc[:, sl],
                                mybir.AluOpType.mult)
                    else:
                        nc.vector.tensor_copy(osD, Ops[HD:P, :])
                        copyO(osO, Ops[0:HD, :])
                        nc.vector.reciprocal_approx_fast(rc, osD)
                        nc.vector.tensor_tensor(
                            OtT[q0 // 1024][hlo:hlo + HD, :],
                            osO, rc, mybir.AluOpType.mult)
                    return (osO, osD, rc)

                oproj_queue = []
                blocks = [(b, nqb, h)
                          for b in range(2) for nqb in range(2)
                          for h in range(2)]
                pend = []       # (pv_fn, chunk_idx, Pt) pipeline carry-over
                prev_ctx = None  # (hlo, q0, Ops, bi) awaiting normalize
                for bi, (b, nqb, h) in enumerate(blocks):
                    q0 = b * NB + nqb * 1024
                    hlo = h * HD
                    Vaug = VaugA if h == 0 else VaugB
                    Kz = KzA if h == 0 else KzB

                    def s_exp(i, b=b, q0=q0, Kz=Kz):
                        k0 = b * NB + i * P
                        Sps = spS.tile([P, 1024], F32, tag="S")
                        if S_SINGLE:
                            nc.tensor.matmul(
                                Sps, Kz[:, k0:k0 + P],
                                Qb[:, ds(q0, 1024)],
                                start=True, stop=True)
                        else:
                            for hf in range(2):
                                nc.tensor.matmul(
                                    Sps[:, ts(hf, 512)],
                                    Kz[:, k0:k0 + P],
                                    Qb[:, ds(q0 + hf * 512, 512)],
                                    start=True, stop=True)
                        Pt = ptp.tile([P, 1024], BF16, tag="P")
                        nc.scalar.activation(
                            Pt, Sps, mybir.ActivationFunctionType.Exp)
                        return Pt

                    # pipeline carries across block boundaries: this
                    # block's first DEPTH S/exp chunks interleave with the
                    # previous block's tail PVs and its normalize.
                    DEPTH = 6
                    first_pts = []
                    for k in range(DEPTH):
                        first_pts.append(s_exp(k))
                        if pend:
                            f, idx, pt = pend.pop(0)
                            f(idx, pt)
                    if prev_ctx is not None:
                        phlo, pq0, pOps, pbi = prev_ctx
                        normalize(phlo, pq0, pOps)
                        if pbi % 2 == 1:
                            for tch in range(8):
                                oproj_queue.append((pq0, tch, pbi))

                    Ops = spO.tile([P, 1024], F32, tag="O")

                    def pv(i, Pt, Vaug=Vaug, b=b, Ops=Ops):
                        kc = b * 16 + i
                        if PV_SINGLE:
                            nc.tensor.matmul(
                                Ops, Vaug[:, kc, :], Pt,
                                start=(i == 0), stop=(i == 15),
                                skip_group_check=True)
                        else:
                            for hf in range(2):
                                nc.tensor.matmul(
                                    Ops[:, ts(hf, 512)],
                                    Vaug[:, kc, :],
                                    Pt[:, ts(hf, 512)],
                                    start=(i == 0), stop=(i == 15),
                                    skip_group_check=True)

                    pend = [(pv, k, first_pts[k]) for k in range(DEPTH)]
                    for i in range(DEPTH, 16):
                        pend.append((pv, i, s_exp(i)))
                        f, idx, pt = pend.pop(0)
                        f(idx, pt)
                        if i % 2 == 0 and oproj_queue:
                            src = oproj_queue[0]
                            if bi - src[2] >= 2:
                                oproj_queue.pop(0)
                                oproj_tile(src[0], src[1])
                    prev_ctx = (hlo, q0, Ops, bi)
                # drain the last block's pipeline + normalize
                for f, idx, pt in pend:
                    f(idx, pt)
                phlo, pq0, pOps, pbi = prev_ctx
                nrm_tiles = normalize(phlo, pq0, pOps, last=True)
                # Keep-warm matmuls bridge the PE gap while the final
                # normalize chain runs (the HAM clock gate throttles the
                # PE to 1.2 GHz after idle windows).  The first batch is
                # dependency-free; later ones consume the normalize
                # intermediates so they fire progressively through the
                # chain instead of all at once.
                for _ in range(6):
                    dmy = spP.tile([P, 512], F32, tag="oproj")
                    nc.tensor.matmul(dmy, wos[:, 0:P], wos[:, 0:512],
                                     start=True, stop=True,
                                     skip_group_check=True)
                for dep in (nrm_tiles[1][:, 0:512], nrm_tiles[1][:, 512:],
                            nrm_tiles[2][:, 0:512], nrm_tiles[2][:, 512:]):
                    dmy = spP.tile([P, 512], F32, tag="oproj")
                    nc.tensor.matmul(dmy, ident[0:HD], dep,
                                     start=True, stop=True,
                                     skip_group_check=True)
                for tch in range(8):
                    oproj_queue.append((pq0, tch, pbi))
                # the drain's PE duty cycle is ~65% (matmuls wait on the
                # PSUM bounce copies), low enough for the clock gate to
                # drop to 1.2 GHz; one dependency-free matmul per chunk
                # into the now-idle O bank keeps the array hot.
                dmyk = spO.tile([P, 512], F32, tag="O", name="dmyk")
                for q0_, tch_, _ in oproj_queue:
                    for _ in range(2):
                        nc.tensor.matmul(dmyk, wos[:, 0:P],
                                         wos[:, 0:512],
                                         start=True, stop=True,
                                         skip_group_check=True)
                    oproj_tile(q0_, tch_, use_act=True)

    nc.compile()
    return nc


def _get_nc():
    global _NC_CACHE
    if _NC_CACHE is None:
        _NC_CACHE = build_nc()
    return _NC_CACHE


def shard_inputs(x, rope_cos, rope_sin, Wq, bq, Wk, bk, Wv, bv, Wo, bo):
    """Build per-core input maps."""
    # [chunk, p, o, tok]: element (c, p, o, t) = x^T[o*128+p, c*512+t]
    xt = np.ascontiguousarray(
        x.reshape(NT // 512, 512, 8, P).transpose(0, 3, 2, 1)
    ).astype(np.float32)
    cosT = np.ascontiguousarray(rope_cos.reshape(NT, HD).T).astype(np.float32)
    sinT = np.ascontiguousarray(rope_sin.reshape(NT, HD).T).astype(np.float32)
    cos_id = np.ones((HD, NT), np.float32)
    sin_id = np.zeros((HD, NT), np.float32)
    # rotate_half as matrix R: out = R @ t, R[2i,2i+1]=-1, R[2i+1,2i]=+1.
    # matmul computes lhsT.T @ rhs, so pass R.T.
    R = np.zeros((P, P), np.float32)
    idx = np.arange(0, HD, 2)
    R[idx, idx + 1] = -1.0
    R[idx + 1, idx] = 1.0
    rmat = np.ascontiguousarray(R.T)

    in_maps = []
    for c in range(N_CORES):
        lo, hi = c * P, (c + 1) * P
        in_maps.append({
            "xt": xt,
            "wq": np.ascontiguousarray(Wq[:, lo:hi]).astype(np.float32),
            "wk": np.ascontiguousarray(Wk[:, lo:hi]).astype(np.float32),
            "wv": np.ascontiguousarray(Wv[:, lo:hi]).astype(np.float32),
            "wo": np.ascontiguousarray(Wo[lo:hi, :]).astype(ml_dtypes.bfloat16),
            "bias": np.ascontiguousarray(
                np.stack([bq[lo:hi], bk[lo:hi], bv[lo:hi]], axis=1)
            ).astype(np.float32),
            "cos": (cosT if c == 0 else cos_id).astype(ml_dtypes.bfloat16),
            "sin": (sinT if c == 0 else sin_id).astype(ml_dtypes.bfloat16),
            "rmat": rmat,
        })
    return in_maps


def run_device(inputs, trace=False, **kw):
    nc = _get_nc()
    in_maps = shard_inputs(**inputs)
    res = run_bass_kernel_spmd(nc, in_maps, core_ids=list(range(N_CORES)),
                               trace=trace, **kw)
    return res


def gather(res, bo):
    acc = res.results[0]["out"].astype(np.float32).copy()
    for c in range(1, N_CORES):
        acc += res.results[c]["out"]
    acc += bo[None, :].astype(np.float32)
    return acc.reshape(2, NB, HID)


def kernel(**inputs):
    # NRT_EXEC_UNIT_UNRECOVERABLE crashes are transient on this fleet;
    # one retry rescues the run.
    try:
        res = run_device(inputs, trace=False)
    except Exception:
        res = run_device(inputs, trace=False)
    return gather(res, np.asarray(inputs["bo"], np.float32))
